# revision 4
# baseline (speedup 1.0000x reference)
"""Multi-head attention block (qkv proj + softmax attention + out proj) on 8
TRN2 NeuronCores, data-parallel over the batch dimension (2 batches/core).

Reference computation (B=16, N=1024, C=1024, H=16, D=64):
    qkv = x @ w_qkv.T                    # [B,N,3C]
    q,k,v per head; attn = softmax(q k^T / sqrt(D)); out = attn @ v
    out = concat_heads @ w_proj.T + b_proj

Device-side design (per core, T = 2*1024 tokens):
  Everything is kept "transposed" (channels on partitions) so that softmax
  denominators come for free out of the PV matmul:
    phase 1a: qkT[o, t] = sum_c wqkT[c, o] * xT[c, t]        (o in q|k chans)
    phase 1b: v[t, vc]  = sum_c xT[c, t] * wvT[c, vc]        (v natural layout,
              written into v_ext with a ones column appended per head)
    phase 2 (per batch, head-pair): S_T[key, q] = kT.T @ qT  (K=64 row-packed
              pairs), E = exp(S_T * scale) on ACT,
              pv[d|1, q] = [v_h | 1].T @ E  accumulated over key tiles
              -> row 64 is the softmax denominator;
              normalize: bcast denom via K=1 matmul, reciprocal on DVE,
              attnT[c, t] = pv[d, q] * recip  (fp16)
    phase 3: out[t, o] = sum_c attnT[c, t] * wpT[c, o] + bias
  All matmuls fp16 inputs with fp32 PSUM accumulation; single-bank PSUM tiles.
"""

import numpy as np

B, N, C = 16, 1024, 1024
H, D = 16, 64
SCALE = D ** -0.5
NCORES = 8
B_SH = B // NCORES            # batches per core
T = B_SH * N                  # tokens per core
CT = C // 128                 # 128-channel tiles per C
PAIRS = H // 2                # head pairs (2 heads share a 128-row tile)
KT = N // 128                 # key tiles per batch
QB = N // 512                 # q blocks of 512 per batch

_CACHE = {}


def _build():
    import concourse.mybir as mybir
    import concourse.tile as tile
    from concourse import bacc

    fp16 = mybir.dt.float16
    fp32 = mybir.dt.float32
    Exp = mybir.ActivationFunctionType.Exp

    nc = bacc.Bacc("TRN2", target_bir_lowering=False, debug=False)

    xT = nc.dram_tensor("xT", [C, T], fp16, kind="ExternalInput")
    wqkT = nc.dram_tensor("wqkT", [C, 2 * C], fp16, kind="ExternalInput")
    wvT = nc.dram_tensor("wvT", [C, C], fp16, kind="ExternalInput")
    wpT = nc.dram_tensor("wpT", [C, C], fp16, kind="ExternalInput")
    bias = nc.dram_tensor("bias", [128, C], fp32, kind="ExternalInput")
    out = nc.dram_tensor("out", [T, C], fp32, kind="ExternalOutput")
    qkT = nc.dram_tensor("qkT", [2 * C, T], fp16)      # device scratch

    xT_r = xT.rearrange("(j p) t -> p j t", p=128)
    wqkT_r = wqkT.rearrange("(j p) o -> p j o", p=128)
    wvT_r = wvT.rearrange("(j p) o -> p j o", p=128)
    wpT_r = wpT.rearrange("(j p) o -> p j o", p=128)

    with tile.TileContext(nc) as tc:
        with tc.tile_pool(name="persist", bufs=1) as persist:
            # [tok%128, tok//128, head, d | ones]
            v_ext = persist.tile([128, 2 * KT, H, D + 1], fp16)
            # [c%128, c//128, tok]
            attnT = persist.tile([128, CT, T], fp16)
            wp_sb = persist.tile([128, CT, C], fp16)
            bias_sb = persist.tile([128, C], fp32)
            ones_sb = persist.tile([128, 64], fp32)
            zero_sb = persist.tile([128, 1], fp32)

            nc.sync.dma_start(wp_sb[:], wpT_r)
            nc.sync.dma_start(bias_sb[:], bias[:])
            nc.any.memset(ones_sb[:], 1.0)
            nc.any.memset(zero_sb[:], 0.0)
            nc.any.memset(v_ext[:, :, :, D:D + 1], 1.0)

            # ---------------- phase 1: qkv projection ----------------
            with (
                tc.tile_pool(name="xpool", bufs=1) as xpool,
                tc.tile_pool(name="wstream", bufs=3) as wstream,
                tc.tile_pool(name="evict1", bufs=4) as evict1,
                tc.tile_pool(name="ps1", bufs=4, space="PSUM") as ps1,
            ):
                x_sb = xpool.tile([128, CT, T], fp16)
                nc.sync.dma_start(x_sb[:], xT_r)

                # 1a: qkT[o, t] for o in the 2C q|k channels
                for ot in range(2 * C // 128):
                    wq_sb = wstream.tile([128, CT, 128], fp16, tag="wq")
                    nc.sync.dma_start(wq_sb[:], wqkT_r[:, :, ot * 128:(ot + 1) * 128])
                    for tb in range(T // 512):
                        ps = ps1.tile([128, 512], fp32)
                        for ct in range(CT):
                            nc.tensor.matmul(
                                ps[:], wq_sb[:, ct, :],
                                x_sb[:, ct, tb * 512:(tb + 1) * 512],
                                start=(ct == 0), stop=(ct == CT - 1))
                        qk_sb = evict1.tile([128, 512], fp16, tag="qke")
                        nc.vector.tensor_copy(qk_sb[:], ps[:])
                        nc.sync.dma_start(
                            qkT[ot * 128:(ot + 1) * 128, tb * 512:(tb + 1) * 512],
                            qk_sb[:])

                # 1b: v in natural layout -> v_ext (with ones column)
                for ob in range(C // 512):
                    wv_sb = wstream.tile([128, CT, 512], fp16, tag="wv")
                    nc.sync.dma_start(wv_sb[:], wvT_r[:, :, ob * 512:(ob + 1) * 512])
                    for tt in range(T // 128):
                        ps = ps1.tile([128, 512], fp32)
                        for ct in range(CT):
                            nc.tensor.matmul(
                                ps[:], x_sb[:, ct, tt * 128:(tt + 1) * 128],
                                wv_sb[:, ct, :],
                                start=(ct == 0), stop=(ct == CT - 1))
                        nc.vector.tensor_copy(
                            v_ext[:, tt, ob * 8:(ob + 1) * 8, 0:D],
                            ps[:].rearrange("p (h d) -> p h d", d=D))

            # ---------------- phase 2: attention ----------------
            with (
                tc.tile_pool(name="qk2", bufs=4) as qk2,
                tc.tile_pool(name="epool", bufs=6) as epool,
                tc.tile_pool(name="dnpool", bufs=4) as dnpool,
                tc.tile_pool(name="bcpool", bufs=4) as bcpool,
                tc.tile_pool(name="ps_s", bufs=4, space="PSUM") as ps_s,
                tc.tile_pool(name="ps_pv", bufs=4, space="PSUM") as ps_pv,
            ):
                for b in range(B_SH):
                    for p in range(PAIRS):
                        q_sb = qk2.tile([128, N], fp16, tag="q")
                        k_sb = qk2.tile([128, N], fp16, tag="k")
                        nc.sync.dma_start(
                            q_sb[:], qkT[p * 128:(p + 1) * 128, b * N:(b + 1) * N])
                        nc.sync.dma_start(
                            k_sb[:], qkT[C + p * 128:C + (p + 1) * 128,
                                         b * N:(b + 1) * N])
                        for qb in range(QB):
                            pv = [ps_pv.tile([D + 1, 512], fp32, tag="pv",
                                             name=f"pv{h2}")
                                  for h2 in range(2)]
                            for kt in range(KT):
                                for h2 in range(2):
                                    ho = h2 * 64
                                    s_ps = ps_s.tile([128, 512], fp32, tag="s")
                                    nc.tensor.matmul(
                                        s_ps[:],
                                        k_sb[ho:ho + 64, kt * 128:(kt + 1) * 128],
                                        q_sb[ho:ho + 64, qb * 512:(qb + 1) * 512],
                                        start=True, stop=True)
                                    e_sb = epool.tile([128, 512], fp16, tag="e")
                                    nc.scalar.activation(
                                        e_sb[:], s_ps[:], Exp,
                                        bias=zero_sb[:], scale=SCALE)
                                    nc.tensor.matmul(
                                        pv[h2][:],
                                        v_ext[:, b * KT + kt, 2 * p + h2, :],
                                        e_sb[:],
                                        start=(kt == 0), stop=(kt == KT - 1))
                            for h2 in range(2):
                                h = 2 * p + h2
                                dn = dnpool.tile([1, 512], fp32, tag="dn")
                                nc.vector.tensor_copy(dn[0:1, :], pv[h2][D:D + 1, :])
                                bc_ps = ps_s.tile([64, 512], fp32, tag="s")
                                nc.tensor.matmul(
                                    bc_ps[:], ones_sb[0:1, :], dn[0:1, :],
                                    start=True, stop=True)
                                bc_sb = bcpool.tile([64, 512], fp32, tag="bc")
                                nc.vector.reciprocal(bc_sb[:], bc_ps[:])
                                tsl = slice(b * N + qb * 512, b * N + (qb + 1) * 512)
                                nc.vector.tensor_mul(
                                    attnT[(h % 2) * 64:(h % 2) * 64 + 64,
                                          h // 2, tsl],
                                    pv[h2][0:D, :], bc_sb[:])

            # ---------------- phase 3: output projection ----------------
            with (
                tc.tile_pool(name="outpool", bufs=4) as outpool,
                tc.tile_pool(name="ps3", bufs=4, space="PSUM") as ps3,
            ):
                for tt in range(T // 128):
                    for ob in range(C // 512):
                        ps = ps3.tile([128, 512], fp32)
                        for j in range(CT):
                            nc.tensor.matmul(
                                ps[:], attnT[:, j, tt * 128:(tt + 1) * 128],
                                wp_sb[:, j, ob * 512:(ob + 1) * 512],
                                start=(j == 0), stop=(j == CT - 1))
                        o_sb = outpool.tile([128, 512], fp32, tag="o")
                        nc.vector.tensor_add(
                            o_sb[:], ps[:], bias_sb[:, ob * 512:(ob + 1) * 512])
                        nc.sync.dma_start(
                            out[tt * 128:(tt + 1) * 128, ob * 512:(ob + 1) * 512],
                            o_sb[:])

    nc.compile()
    return nc


def _get_nc():
    if "nc" not in _CACHE:
        _CACHE["nc"] = _build()
    return _CACHE["nc"]


def _prep_inputs(x, w_qkv, w_proj, b_proj):
    x16 = np.ascontiguousarray(x, dtype=np.float16)
    wq16 = np.asarray(w_qkv, dtype=np.float16)
    wp16 = np.asarray(w_proj, dtype=np.float16)
    wqkT_np = np.ascontiguousarray(wq16[0:2 * C].T)          # [C, 2C]
    wvT_np = np.ascontiguousarray(wq16[2 * C:3 * C].T)       # [C, C]
    wpT_np = np.ascontiguousarray(wp16.T)                    # [C, C]
    bias_np = np.ascontiguousarray(
        np.broadcast_to(np.asarray(b_proj, dtype=np.float32)[None, :], (128, C)))
    in_maps = []
    for core in range(NCORES):
        xs = x16[core * B_SH:(core + 1) * B_SH]              # [B_SH, N, C]
        xT_np = np.ascontiguousarray(xs.transpose(2, 0, 1).reshape(C, T))
        in_maps.append({
            "xT": xT_np, "wqkT": wqkT_np, "wvT": wvT_np,
            "wpT": wpT_np, "bias": bias_np,
        })
    return in_maps


def _install_ntff_hook():
    """The agent image's antenv lacks axon_hooks; synthesize it so
    run_bass_kernel_spmd(trace=True) can capture NTFF profiles."""
    import sys
    import types
    try:
        from antenv.axon_hooks import get_axon_ntff_profile_hook  # noqa: F401
        return
    except ImportError:
        pass
    import antenv
    mod = types.ModuleType("antenv.axon_hooks")
    state = {"hook": None}
    mod.set_axon_ntff_profile_hook = lambda h: state.__setitem__("hook", h)
    mod.get_axon_ntff_profile_hook = lambda: state["hook"]
    sys.modules["antenv.axon_hooks"] = mod
    antenv.axon_hooks = mod
    try:
        from trn_agent_boot.trn_boot import _ntff_profile_via_ctypes
        mod.set_axon_ntff_profile_hook(
            _ntff_profile_via_ctypes("/opt/axon/libaxon_pjrt.so"))
    except Exception as e:  # tracing degrades, run still works
        print("ntff hook install failed:", e)


def run(x, w_qkv, w_proj, b_proj, trace=False):
    """Returns (full_output [B,N,C] fp32, BassKernelResults)."""
    from concourse.bass_utils import run_bass_kernel_spmd

    if trace:
        _install_ntff_hook()
    nc = _get_nc()
    in_maps = _prep_inputs(x, w_qkv, w_proj, b_proj)
    res = run_bass_kernel_spmd(
        nc, in_maps, core_ids=list(range(NCORES)), trace=trace)
    out_full = np.concatenate(
        [r["out"].reshape(B_SH, N, C) for r in res.results], axis=0)
    return out_full.astype(np.float32), res


def kernel(x, w_qkv, w_proj, b_proj):
    out_full, _ = run(x, w_qkv, w_proj, b_proj, trace=False)
    return out_full


# revision 7
# speedup vs baseline: 1.4108x; 1.4108x over previous
"""Multi-head attention block (qkv proj + softmax attention + out proj) on 8
TRN2 NeuronCores, data-parallel over the batch dimension (2 batches/core).

Reference computation (B=16, N=1024, C=1024, H=16, D=64):
    qkv = x @ w_qkv.T                    # [B,N,3C]
    q,k,v per head; attn = softmax(q k^T / sqrt(D)); out = attn @ v
    out = concat_heads @ w_proj.T + b_proj

Device-side design (per core, T = 2*1024 tokens), all channels-on-partitions
("transposed") so the softmax denominator falls out of the PV matmul:
    qk:   qkT[o, t]   = sum_c wqkT[c, o] * xT[c, t]     (q|k channels)
    v:    v[t, vc]    = sum_c xT[c, t] * wvT[c, vc]     (+ ones column/head)
    attn: S_T[key, q] = kT.T @ qT  (K=64, head pairs row-packed on the PE),
          E = exp(S_T * scale) on ACT (no max subtraction needed: |S*scale|<6),
          pv[d|1, q] = [v_h | 1].T @ E  accumulated over key tiles
          -> partition 64 of pv is the denominator;
          normalize: denom -> SBUF, broadcast via K=1 matmul,
          reciprocal_approx_fast, attnT = pv * recip  (fp16)
    out:  out[t, o] = sum_c attnT[c, t] * wpT[c, o] + bias
All matmul inputs fp16 with fp32 PSUM accumulation; PSUM tiles single-bank.

The emission order software-pipelines the whole kernel so the tensor engine
never idles on the ACT exp round-trip (idle PE re-throttles to half clock):
    stage A: qk/v projection chains for batch-0 tokens (dense PE warmup)
    stage B: attention for batch 0, with batch-1 projection chains as filler
    stage C: attention for batch 1, with batch-0 output-proj chains as filler
    stage D: remaining batch-1 output-proj chains
Inside attention units the PV matmuls run one key-tile behind the S matmuls
so the exp latency is hidden by the next tile's S work.
"""

import numpy as np

B, N, C = 16, 1024, 1024
H, D = 16, 64
SCALE = D ** -0.5
NCORES = 8
B_SH = B // NCORES            # batches per core
T = B_SH * N                  # tokens per core
CT = C // 128                 # 128-channel tiles per C
PAIRS = H // 2                # head pairs (2 heads share a 128-row tile)
KT = N // 128                 # key tiles per batch
QB = N // 512                 # q blocks of 512 per batch

_CACHE = {}


def _build():
    import concourse.mybir as mybir
    import concourse.tile as tile
    from concourse import bacc

    fp16 = mybir.dt.float16
    fp32 = mybir.dt.float32
    Exp = mybir.ActivationFunctionType.Exp

    nc = bacc.Bacc("TRN2", target_bir_lowering=False, debug=False)

    xT = nc.dram_tensor("xT", [C, T], fp16, kind="ExternalInput")
    wqkT = nc.dram_tensor("wqkT", [C, 2 * C], fp16, kind="ExternalInput")
    wvT = nc.dram_tensor("wvT", [C, C], fp16, kind="ExternalInput")
    wpT = nc.dram_tensor("wpT", [C, C], fp16, kind="ExternalInput")
    bias = nc.dram_tensor("bias", [128, C], fp32, kind="ExternalInput")
    out = nc.dram_tensor("out", [T, C], fp32, kind="ExternalOutput")
    qkT = nc.dram_tensor("qkT", [2 * C, T], fp16)      # device scratch

    xT_r = xT.rearrange("(j p) t -> p j t", p=128)
    wqkT_r = wqkT.rearrange("(j p) o -> p j o", p=128)
    wvT_r = wvT.rearrange("(j p) o -> p j o", p=128)
    wpT_r = wpT.rearrange("(j p) o -> p j o", p=128)

    with tile.TileContext(nc) as tc:
        with (
            tc.tile_pool(name="persist", bufs=1) as persist,
            tc.tile_pool(name="wstream", bufs=3) as wstream,
            tc.tile_pool(name="evict1", bufs=3) as evict1,
            tc.tile_pool(name="qk2", bufs=4) as qk2,
            tc.tile_pool(name="epool", bufs=6) as epool,
            tc.tile_pool(name="dnpool", bufs=4) as dnpool,
            tc.tile_pool(name="bcpool", bufs=4) as bcpool,
            tc.tile_pool(name="outpool", bufs=4) as outpool,
            tc.tile_pool(name="psum", bufs=1, space="PSUM") as psum,
        ):
            # [tok%128, tok//128, head, d | ones]
            v_ext = persist.tile([128, 2 * KT, H, D + 1], fp16)
            # [c%128, c//128, tok]
            attnT = persist.tile([128, CT, T], fp16)
            x_sb = persist.tile([128, CT, T], fp16)
            wv_sb = persist.tile([128, CT, C], fp16)
            wp_sb = persist.tile([128, CT, C], fp16)
            bias_sb = persist.tile([128, C], fp32)
            ones_sb = persist.tile([128, 64], fp32)
            zero_sb = persist.tile([128, 1], fp32)

            nc.sync.dma_start(x_sb[:], xT_r)
            nc.sync.dma_start(wv_sb[:], wvT_r)
            nc.sync.dma_start(wp_sb[:], wpT_r)
            nc.sync.dma_start(bias_sb[:], bias[:])
            nc.any.memset(ones_sb[:], 1.0)
            nc.any.memset(zero_sb[:], 0.0)
            nc.any.memset(v_ext[:, :, :, D:D + 1], 1.0)

            # -------- projection / output work units (PE filler chains) -----
            def qk_unit(ot):
                """qkT[ot*128:(ot+1)*128, :] for half the tokens; which half is
                decided per-chain by the tb list passed in."""
                def emit(tbs):
                    wq_sb = wstream.tile([128, CT, 128], fp16, tag="wq",
                                         name=f"wq_{ot}_{tbs[0]}")
                    nc.sync.dma_start(
                        wq_sb[:], wqkT_r[:, :, ot * 128:(ot + 1) * 128])
                    for tb in tbs:
                        ps = psum.tile([128, 512], fp32, tag="mm", bufs=2,
                                       name=f"qkps_{ot}_{tb}")
                        for ct in range(CT):
                            nc.tensor.matmul(
                                ps[:], wq_sb[:, ct, :],
                                x_sb[:, ct, tb * 512:(tb + 1) * 512],
                                start=(ct == 0), stop=(ct == CT - 1))
                        qk_sb = evict1.tile([128, 512], fp16, tag="qke",
                                            name=f"qke_{ot}_{tb}")
                        nc.vector.tensor_copy(qk_sb[:], ps[:])
                        nc.sync.dma_start(
                            qkT[ot * 128:(ot + 1) * 128,
                                tb * 512:(tb + 1) * 512], qk_sb[:])
                return emit

            def v_chain(tt, ob):
                ps = psum.tile([128, 512], fp32, tag="mm", bufs=2,
                               name=f"vps_{tt}_{ob}")
                for ct in range(CT):
                    nc.tensor.matmul(
                        ps[:], x_sb[:, ct, tt * 128:(tt + 1) * 128],
                        wv_sb[:, ct, ob * 512:(ob + 1) * 512],
                        start=(ct == 0), stop=(ct == CT - 1))
                nc.vector.tensor_copy(
                    v_ext[:, tt, ob * 8:(ob + 1) * 8, 0:D],
                    ps[:].rearrange("p (h d) -> p h d", d=D))

            def out_chain(tt, ob):
                ps = psum.tile([128, 512], fp32, tag="mm", bufs=2,
                               name=f"ops_{tt}_{ob}")
                for j in range(CT):
                    nc.tensor.matmul(
                        ps[:], attnT[:, j, tt * 128:(tt + 1) * 128],
                        wp_sb[:, j, ob * 512:(ob + 1) * 512],
                        start=(j == 0), stop=(j == CT - 1))
                o_sb = outpool.tile([128, 512], fp32, tag="o",
                                    name=f"osb_{tt}_{ob}")
                nc.vector.tensor_add(
                    o_sb[:], ps[:], bias_sb[:, ob * 512:(ob + 1) * 512])
                nc.sync.dma_start(
                    out[tt * 128:(tt + 1) * 128, ob * 512:(ob + 1) * 512],
                    o_sb[:])

            # -------- attention unit (one batch, one head pair) ------------
            def attn_unit(b, p, fillers, max_fill):
                filled = [0]

                def maybe_fill():
                    if fillers and filled[0] < max_fill:
                        fillers.pop(0)()
                        filled[0] += 1

                q_sb = qk2.tile([128, N], fp16, tag="q", name=f"q_{b}_{p}")
                k_sb = qk2.tile([128, N], fp16, tag="k", name=f"k_{b}_{p}")
                nc.sync.dma_start(
                    q_sb[:], qkT[p * 128:(p + 1) * 128, b * N:(b + 1) * N])
                nc.sync.dma_start(
                    k_sb[:], qkT[C + p * 128:C + (p + 1) * 128,
                                 b * N:(b + 1) * N])
                for qb in range(QB):
                    pv = [psum.tile([D + 1, 512], fp32, tag="pv", bufs=3,
                                    name=f"pv_{b}_{p}_{qb}_{h2}")
                          for h2 in range(2)]
                    e_tiles = {}
                    for kt in range(KT):
                        for h2 in range(2):
                            ho = h2 * 64
                            s_ps = psum.tile([128, 512], fp32, tag="s", bufs=3,
                                             name=f"s_{b}_{p}_{qb}_{kt}_{h2}")
                            nc.tensor.matmul(
                                s_ps[:],
                                k_sb[ho:ho + 64, kt * 128:(kt + 1) * 128],
                                q_sb[ho:ho + 64, qb * 512:(qb + 1) * 512],
                                start=True, stop=True)
                            e_sb = epool.tile([128, 512], fp16, tag="e",
                                              name=f"e_{b}_{p}_{qb}_{kt}_{h2}")
                            nc.scalar.activation(
                                e_sb[:], s_ps[:], Exp,
                                bias=zero_sb[:], scale=SCALE)
                            e_tiles[(kt, h2)] = e_sb
                        if kt > 0:
                            for h2 in range(2):
                                nc.tensor.matmul(
                                    pv[h2][:],
                                    v_ext[:, b * KT + kt - 1, 2 * p + h2, :],
                                    e_tiles.pop((kt - 1, h2)),
                                    start=(kt == 1), stop=False)
                        if kt in (2, 5):
                            maybe_fill()
                    for h2 in range(2):
                        nc.tensor.matmul(
                            pv[h2][:],
                            v_ext[:, b * KT + KT - 1, 2 * p + h2, :],
                            e_tiles.pop((KT - 1, h2)),
                            start=False, stop=True)
                    for h2 in range(2):
                        h = 2 * p + h2
                        dn = dnpool.tile([1, 512], fp32, tag="dn",
                                         name=f"dn_{b}_{p}_{qb}_{h2}")
                        nc.vector.tensor_copy(dn[0:1, :], pv[h2][D:D + 1, :])
                        bc_ps = psum.tile([64, 512], fp32, tag="s", bufs=3,
                                          name=f"bcps_{b}_{p}_{qb}_{h2}")
                        nc.tensor.matmul(
                            bc_ps[:], ones_sb[0:1, :], dn[0:1, :],
                            start=True, stop=True)
                        bc_sb = bcpool.tile([64, 512], fp32, tag="bc",
                                            name=f"bc_{b}_{p}_{qb}_{h2}")
                        nc.vector.reciprocal_approx_fast(bc_sb[:], bc_ps[:])
                        tsl = slice(b * N + qb * 512, b * N + (qb + 1) * 512)
                        nc.vector.tensor_mul(
                            attnT[h2 * 64:h2 * 64 + 64, p, tsl],
                            pv[h2][0:D, :], bc_sb[:])
                    maybe_fill()

            # -------------------- emission schedule -----------------------
            # stage A: batch-0 projections (dense PE warmup)
            for ot in range(2 * CT):
                qk_unit(ot)([0, 1])
            for tt in range(KT):
                for ob in range(2):
                    v_chain(tt, ob)

            # stage B: attention b=0, filler = batch-1 projections
            fillers = [lambda ot=ot: qk_unit(ot)([2, 3]) for ot in range(2 * CT)]
            fillers += [lambda tt=tt, ob=ob: v_chain(tt, ob)
                        for tt in range(KT, 2 * KT) for ob in range(2)]
            for p in range(PAIRS):
                attn_unit(0, p, fillers, max_fill=4)
            while fillers:
                fillers.pop(0)()

            # stage C: attention b=1, filler = batch-0 output projection
            fillers = [lambda tt=tt, ob=ob: out_chain(tt, ob)
                       for tt in range(KT) for ob in range(2)]
            for p in range(PAIRS):
                attn_unit(1, p, fillers, max_fill=2)
            while fillers:
                fillers.pop(0)()

            # stage D: batch-1 output projection
            for tt in range(KT, 2 * KT):
                for ob in range(2):
                    out_chain(tt, ob)

    nc.compile()
    return nc


def _get_nc():
    if "nc" not in _CACHE:
        _CACHE["nc"] = _build()
    return _CACHE["nc"]


def _prep_inputs(x, w_qkv, w_proj, b_proj):
    x16 = np.ascontiguousarray(x, dtype=np.float16)
    wq16 = np.asarray(w_qkv, dtype=np.float16)
    wp16 = np.asarray(w_proj, dtype=np.float16)
    wqkT_np = np.ascontiguousarray(wq16[0:2 * C].T)          # [C, 2C]
    wvT_np = np.ascontiguousarray(wq16[2 * C:3 * C].T)       # [C, C]
    wpT_np = np.ascontiguousarray(wp16.T)                    # [C, C]
    bias_np = np.ascontiguousarray(
        np.broadcast_to(np.asarray(b_proj, dtype=np.float32)[None, :], (128, C)))
    in_maps = []
    for core in range(NCORES):
        xs = x16[core * B_SH:(core + 1) * B_SH]              # [B_SH, N, C]
        xT_np = np.ascontiguousarray(xs.transpose(2, 0, 1).reshape(C, T))
        in_maps.append({
            "xT": xT_np, "wqkT": wqkT_np, "wvT": wvT_np,
            "wpT": wpT_np, "bias": bias_np,
        })
    return in_maps


def _install_ntff_hook():
    """The agent image's antenv lacks axon_hooks; synthesize it so
    run_bass_kernel_spmd(trace=True) can capture NTFF profiles."""
    import sys
    import types
    try:
        from antenv.axon_hooks import get_axon_ntff_profile_hook  # noqa: F401
        return
    except ImportError:
        pass
    import antenv
    mod = types.ModuleType("antenv.axon_hooks")
    state = {"hook": None}
    mod.set_axon_ntff_profile_hook = lambda h: state.__setitem__("hook", h)
    mod.get_axon_ntff_profile_hook = lambda: state["hook"]
    sys.modules["antenv.axon_hooks"] = mod
    antenv.axon_hooks = mod
    try:
        from trn_agent_boot.trn_boot import _ntff_profile_via_ctypes
        mod.set_axon_ntff_profile_hook(
            _ntff_profile_via_ctypes("/opt/axon/libaxon_pjrt.so"))
    except Exception as e:  # tracing degrades, run still works
        print("ntff hook install failed:", e)


def run(x, w_qkv, w_proj, b_proj, trace=False):
    """Returns (full_output [B,N,C] fp32, BassKernelResults)."""
    from concourse.bass_utils import run_bass_kernel_spmd

    if trace:
        _install_ntff_hook()
    nc = _get_nc()
    in_maps = _prep_inputs(x, w_qkv, w_proj, b_proj)
    res = run_bass_kernel_spmd(
        nc, in_maps, core_ids=list(range(NCORES)), trace=trace)
    out_full = np.concatenate(
        [r["out"].reshape(B_SH, N, C) for r in res.results], axis=0)
    return out_full.astype(np.float32), res


def kernel(x, w_qkv, w_proj, b_proj):
    out_full, _ = run(x, w_qkv, w_proj, b_proj, trace=False)
    return out_full


# revision 11
# speedup vs baseline: 1.6850x; 1.1943x over previous
"""Multi-head attention block (qkv proj + softmax attention + out proj) on 8
TRN2 NeuronCores, data-parallel over the batch dimension (2 batches/core).

Reference computation (B=16, N=1024, C=1024, H=16, D=64):
    qkv = x @ w_qkv.T                    # [B,N,3C]
    q,k,v per head; attn = softmax(q k^T / sqrt(D)); out = attn @ v
    out = concat_heads @ w_proj.T + b_proj

Device-side design (per core, T = 2*1024 tokens), all channels-on-partitions
("transposed") so the softmax denominator falls out of the PV matmul:
    qk:   qkT[o, t]   = sum_c wqkT[c, o] * xT[c, t]     (q|k channels)
    v:    v[t, vc]    = sum_c xT[c, t] * wvT[c, vc]     (+ ones column/head)
    attn: S_T[key, q] = kT.T @ qT  (K=64, head pairs row-packed on the PE),
          E = exp(S_T * scale) on ACT (no max subtraction needed: |S*scale|<6),
          pv[d|1, q] = [v_h | 1].T @ E  accumulated over key tiles
          -> partition 64 of pv is the denominator;
          normalize: denom -> SBUF, broadcast via K=1 matmul,
          reciprocal_approx_fast, attnT = pv * recip  (fp16)
    out:  out[t, o] = sum_c attnT[c, t] * wpT[c, o] + bias
All matmul inputs fp16 with fp32 PSUM accumulation; PSUM tiles single-bank.

The emission order software-pipelines the whole kernel so the tensor engine
never idles on the ACT exp round-trip (idle PE re-throttles to half clock):
    stage A: qk/v projection chains for batch-0 tokens (dense PE warmup)
    stage B: attention for batch 0, with batch-1 projection chains as filler
    stage C: attention for batch 1, with batch-0 output-proj chains as filler
    stage D: remaining batch-1 output-proj chains
Inside attention units the PV matmuls run one key-tile behind the S matmuls
so the exp latency is hidden by the next tile's S work.
"""

import numpy as np

B, N, C = 16, 1024, 1024
H, D = 16, 64
SCALE = D ** -0.5
NCORES = 8
B_SH = B // NCORES            # batches per core
T = B_SH * N                  # tokens per core
CT = C // 128                 # 128-channel tiles per C
PAIRS = H // 2                # head pairs (2 heads share a 128-row tile)
KT = N // 128                 # key tiles per batch
QB = N // 512                 # q blocks of 512 per batch

_CACHE = {}


def _build():
    import concourse.mybir as mybir
    import concourse.tile as tile
    from concourse import bacc

    fp16 = mybir.dt.float16
    fp32 = mybir.dt.float32
    Exp = mybir.ActivationFunctionType.Exp

    nc = bacc.Bacc("TRN2", target_bir_lowering=False, debug=False)

    xT = nc.dram_tensor("xT", [C, T], fp16, kind="ExternalInput")
    wqkT = nc.dram_tensor("wqkT", [C, 2 * C], fp16, kind="ExternalInput")
    wvT = nc.dram_tensor("wvT", [C, C], fp16, kind="ExternalInput")
    wpT = nc.dram_tensor("wpT", [C, C], fp16, kind="ExternalInput")
    bias = nc.dram_tensor("bias", [128, C], fp32, kind="ExternalInput")
    out = nc.dram_tensor("out", [T, C], fp32, kind="ExternalOutput")
    qkT = nc.dram_tensor("qkT", [2 * C, T], fp16)      # device scratch

    xT_r = xT.rearrange("(j p) t -> p j t", p=128)
    wqkT_r = wqkT.rearrange("(j p) o -> p j o", p=128)
    wvT_r = wvT.rearrange("(j p) o -> p j o", p=128)
    wpT_r = wpT.rearrange("(j p) o -> p j o", p=128)

    with tile.TileContext(nc) as tc:
        with (
            tc.tile_pool(name="persist", bufs=1) as persist,
            tc.tile_pool(name="wstream", bufs=3) as wstream,
            tc.tile_pool(name="evict1", bufs=3) as evict1,
            tc.tile_pool(name="qk2", bufs=4) as qk2,
            tc.tile_pool(name="epool", bufs=6) as epool,
            tc.tile_pool(name="dnpool", bufs=4) as dnpool,
            tc.tile_pool(name="bcpool", bufs=4) as bcpool,
            tc.tile_pool(name="outpool", bufs=4) as outpool,
            tc.tile_pool(name="psum", bufs=1, space="PSUM") as psum,
        ):
            # [tok%128, tok//128, head, d | ones]
            v_ext = persist.tile([128, 2 * KT, H, D + 1], fp16)
            # [c%128, c//128, tok]
            attnT = persist.tile([128, CT, T], fp16)
            x_sb = persist.tile([128, CT, T], fp16)
            wv_sb = persist.tile([128, CT, C], fp16)
            wp_sb = persist.tile([128, CT, C], fp16)
            bias_sb = persist.tile([128, C], fp32)
            ones_sb = persist.tile([128, 64], fp32)
            zero_sb = persist.tile([128, 1], fp32)

            nc.sync.dma_start(x_sb[:], xT_r)
            nc.sync.dma_start(wv_sb[:], wvT_r)
            nc.sync.dma_start(wp_sb[:], wpT_r)
            nc.sync.dma_start(bias_sb[:], bias[:])
            nc.any.memset(ones_sb[:], 1.0)
            nc.any.memset(zero_sb[:], 0.0)
            nc.any.memset(v_ext[:, :, :, D:D + 1], 1.0)

            # -------- projection / output work units (PE filler chains) -----
            def qk_load(ot, stage):
                wq_sb = wstream.tile([128, CT, 128], fp16, tag="wq",
                                     name=f"wq_{ot}_{stage}")
                nc.sync.dma_start(
                    wq_sb[:], wqkT_r[:, :, ot * 128:(ot + 1) * 128])
                return wq_sb

            def qk_unit(ot):
                """qkT[ot*128:(ot+1)*128, :] for half the tokens; which half is
                decided per-chain by the tb list passed in."""
                def emit(tbs, wq_sb=None):
                    if wq_sb is None:
                        wq_sb = qk_load(ot, tbs[0])
                    for tb in tbs:
                        ps = psum.tile([128, 512], fp32, tag="mm", bufs=2,
                                       name=f"qkps_{ot}_{tb}")
                        for ct in range(CT):
                            nc.tensor.matmul(
                                ps[:], wq_sb[:, ct, :],
                                x_sb[:, ct, tb * 512:(tb + 1) * 512],
                                start=(ct == 0), stop=(ct == CT - 1))
                        qk_sb = evict1.tile([128, 512], fp16, tag="qke",
                                            name=f"qke_{ot}_{tb}")
                        nc.vector.tensor_copy(qk_sb[:], ps[:])
                        nc.sync.dma_start(
                            qkT[ot * 128:(ot + 1) * 128,
                                tb * 512:(tb + 1) * 512], qk_sb[:])
                return emit

            def v_chain(tt, ob):
                ps = psum.tile([128, 512], fp32, tag="mm", bufs=2,
                               name=f"vps_{tt}_{ob}")
                for ct in range(CT):
                    nc.tensor.matmul(
                        ps[:], x_sb[:, ct, tt * 128:(tt + 1) * 128],
                        wv_sb[:, ct, ob * 512:(ob + 1) * 512],
                        start=(ct == 0), stop=(ct == CT - 1))
                nc.vector.tensor_copy(
                    v_ext[:, tt, ob * 8:(ob + 1) * 8, 0:D],
                    ps[:].rearrange("p (h d) -> p h d", d=D))

            def out_chain(tt, ob):
                ps = psum.tile([128, 512], fp32, tag="mm", bufs=2,
                               name=f"ops_{tt}_{ob}")
                for j in range(CT):
                    nc.tensor.matmul(
                        ps[:], attnT[:, j, tt * 128:(tt + 1) * 128],
                        wp_sb[:, j, ob * 512:(ob + 1) * 512],
                        start=(j == 0), stop=(j == CT - 1))
                o_sb = outpool.tile([128, 512], fp32, tag="o",
                                    name=f"osb_{tt}_{ob}")
                nc.vector.tensor_add(
                    o_sb[:], ps[:], bias_sb[:, ob * 512:(ob + 1) * 512])
                nc.sync.dma_start(
                    out[tt * 128:(tt + 1) * 128, ob * 512:(ob + 1) * 512],
                    o_sb[:])

            # -------- attention unit (one batch, one head pair) ------------
            def attn_load(b, p):
                q_sb = qk2.tile([128, N], fp16, tag="q", name=f"q_{b}_{p}")
                k_sb = qk2.tile([128, N], fp16, tag="k", name=f"k_{b}_{p}")
                nc.sync.dma_start(
                    q_sb[:], qkT[p * 128:(p + 1) * 128, b * N:(b + 1) * N])
                nc.sync.dma_start(
                    k_sb[:], qkT[C + p * 128:C + (p + 1) * 128,
                                 b * N:(b + 1) * N])
                return q_sb, k_sb

            def attn_unit(b, p, q_sb, k_sb, fillers, max_fill):
                filled = [0]

                def maybe_fill():
                    if fillers and filled[0] < max_fill:
                        fillers.pop()
                        filled[0] += 1
                for qb in range(QB):
                    pv = [psum.tile([D + 1, 512], fp32, tag="pv", bufs=2,
                                    name=f"pv_{b}_{p}_{qb}_{h2}")
                          for h2 in range(2)]
                    e_tiles = {}
                    for kt in range(KT):
                        for h2 in range(2):
                            ho = h2 * 64
                            s_ps = psum.tile([128, 512], fp32, tag="s", bufs=4,
                                             name=f"s_{b}_{p}_{qb}_{kt}_{h2}")
                            nc.tensor.matmul(
                                s_ps[:],
                                k_sb[ho:ho + 64, kt * 128:(kt + 1) * 128],
                                q_sb[ho:ho + 64, qb * 512:(qb + 1) * 512],
                                start=True, stop=True)
                            e_sb = epool.tile([128, 512], fp16, tag="e",
                                              name=f"e_{b}_{p}_{qb}_{kt}_{h2}")
                            nc.scalar.activation(
                                e_sb[:], s_ps[:], Exp,
                                bias=zero_sb[:], scale=SCALE)
                            e_tiles[(kt, h2)] = e_sb
                        if kt > 0:
                            for h2 in range(2):
                                nc.tensor.matmul(
                                    pv[h2][:],
                                    v_ext[:, b * KT + kt - 1, 2 * p + h2, :],
                                    e_tiles.pop((kt - 1, h2)),
                                    start=(kt == 1), stop=False)
                        if kt in (2, 5):
                            maybe_fill()
                    for h2 in range(2):
                        nc.tensor.matmul(
                            pv[h2][:],
                            v_ext[:, b * KT + KT - 1, 2 * p + h2, :],
                            e_tiles.pop((KT - 1, h2)),
                            start=False, stop=True)
                    for h2 in range(2):
                        dn_raw = dnpool.tile([1, 512], fp32, tag="dnr",
                                             name=f"dnr_{b}_{p}_{qb}_{h2}")
                        nc.vector.tensor_copy(dn_raw[0:1, :], pv[h2][D:D + 1, :])
                        dn = dnpool.tile([1, 512], fp32, tag="dn",
                                         name=f"dn_{b}_{p}_{qb}_{h2}")
                        nc.vector.reciprocal_approx_fast(
                            dn[0:1, :], dn_raw[0:1, :])
                        bc_sb = bcpool.tile([64, 512], fp32, tag="bc",
                                            name=f"bc_{b}_{p}_{qb}_{h2}")
                        nc.gpsimd.partition_broadcast(
                            bc_sb[:], dn[0:1, :], channels=64)
                        tsl = slice(b * N + qb * 512, b * N + (qb + 1) * 512)
                        nc.vector.tensor_mul(
                            attnT[h2 * 64:h2 * 64 + 64, p, tsl],
                            pv[h2][0:D, :], bc_sb[:])
                    maybe_fill()

            # -------------------- emission schedule -----------------------
            class FillerQueue:
                """Filler work as (load_fn, compute_fn) pairs; loads (weight
                DMAs) are issued `depth` items ahead of compute so inserted
                chains never stall the PE on their own DMA."""

                def __init__(self, items, depth=2):
                    self.items = items
                    self.loaded = {}
                    self.next_load = 0
                    self.next_compute = 0
                    self.depth = depth
                    self._pump()

                def _pump(self):
                    while (self.next_load < len(self.items)
                           and self.next_load < self.next_compute + self.depth):
                        load_fn = self.items[self.next_load][0]
                        self.loaded[self.next_load] = (
                            load_fn() if load_fn else None)
                        self.next_load += 1

                def pop(self):
                    if self.next_compute >= len(self.items):
                        return
                    i = self.next_compute
                    self.items[i][1](self.loaded.pop(i))
                    self.next_compute += 1
                    self._pump()

                def __bool__(self):
                    return self.next_compute < len(self.items)

                def flush(self):
                    while self:
                        self.pop()

            # stage A: batch-0 projections (dense PE warmup)
            for ot in range(2 * CT):
                qk_unit(ot)([0, 1])
            for tt in range(KT):
                for ob in range(2):
                    v_chain(tt, ob)

            # stage B filler: batch-1 projections (qk units interleaved with
            # v chains). stage C filler: batch-0 output projection.
            items_b = []
            for i in range(2 * CT):
                items_b.append((lambda ot=i: qk_load(ot, 23),
                                lambda wq, ot=i: qk_unit(ot)([2, 3], wq)))
                if i < KT:
                    items_b.append((None, lambda _, tt=KT + i: v_chain(tt, 0)))
                    items_b.append((None, lambda _, tt=KT + i: v_chain(tt, 1)))
            fq_b = FillerQueue(items_b)
            fq_c = FillerQueue([
                (None, lambda _, tt=tt, ob=ob: out_chain(tt, ob))
                for tt in range(KT) for ob in range(2)])

            units = [(0, p) for p in range(PAIRS)] + [(1, p) for p in range(PAIRS)]
            qk_tiles = {0: attn_load(*units[0])}
            for i, (b, p) in enumerate(units):
                if i + 1 < len(units):
                    qk_tiles[i + 1] = attn_load(*units[i + 1])
                if b == 0:
                    attn_unit(b, p, *qk_tiles.pop(i), fq_b, max_fill=4)
                else:
                    attn_unit(b, p, *qk_tiles.pop(i), fq_c, max_fill=2)
                if i == PAIRS - 1:
                    fq_b.flush()
            fq_c.flush()

            # stage D: batch-1 output projection
            for tt in range(KT, 2 * KT):
                for ob in range(2):
                    out_chain(tt, ob)

    nc.compile()
    return nc


def _get_nc():
    if "nc" not in _CACHE:
        _CACHE["nc"] = _build()
    return _CACHE["nc"]


def _prep_inputs(x, w_qkv, w_proj, b_proj):
    x16 = np.ascontiguousarray(x, dtype=np.float16)
    wq16 = np.asarray(w_qkv, dtype=np.float16)
    wp16 = np.asarray(w_proj, dtype=np.float16)
    wqkT_np = np.ascontiguousarray(wq16[0:2 * C].T)          # [C, 2C]
    wvT_np = np.ascontiguousarray(wq16[2 * C:3 * C].T)       # [C, C]
    wpT_np = np.ascontiguousarray(wp16.T)                    # [C, C]
    bias_np = np.ascontiguousarray(
        np.broadcast_to(np.asarray(b_proj, dtype=np.float32)[None, :], (128, C)))
    in_maps = []
    for core in range(NCORES):
        xs = x16[core * B_SH:(core + 1) * B_SH]              # [B_SH, N, C]
        xT_np = np.ascontiguousarray(xs.transpose(2, 0, 1).reshape(C, T))
        in_maps.append({
            "xT": xT_np, "wqkT": wqkT_np, "wvT": wvT_np,
            "wpT": wpT_np, "bias": bias_np,
        })
    return in_maps


def _install_ntff_hook():
    """The agent image's antenv lacks axon_hooks; synthesize it so
    run_bass_kernel_spmd(trace=True) can capture NTFF profiles."""
    import sys
    import types
    try:
        from antenv.axon_hooks import get_axon_ntff_profile_hook  # noqa: F401
        return
    except ImportError:
        pass
    import antenv
    mod = types.ModuleType("antenv.axon_hooks")
    state = {"hook": None}
    mod.set_axon_ntff_profile_hook = lambda h: state.__setitem__("hook", h)
    mod.get_axon_ntff_profile_hook = lambda: state["hook"]
    sys.modules["antenv.axon_hooks"] = mod
    antenv.axon_hooks = mod
    try:
        from trn_agent_boot.trn_boot import _ntff_profile_via_ctypes
        mod.set_axon_ntff_profile_hook(
            _ntff_profile_via_ctypes("/opt/axon/libaxon_pjrt.so"))
    except Exception as e:  # tracing degrades, run still works
        print("ntff hook install failed:", e)


def run(x, w_qkv, w_proj, b_proj, trace=False):
    """Returns (full_output [B,N,C] fp32, BassKernelResults)."""
    from concourse.bass_utils import run_bass_kernel_spmd

    if trace:
        _install_ntff_hook()
    nc = _get_nc()
    in_maps = _prep_inputs(x, w_qkv, w_proj, b_proj)
    res = run_bass_kernel_spmd(
        nc, in_maps, core_ids=list(range(NCORES)), trace=trace)
    out_full = np.concatenate(
        [r["out"].reshape(B_SH, N, C) for r in res.results], axis=0)
    return out_full.astype(np.float32), res


def kernel(x, w_qkv, w_proj, b_proj):
    out_full, _ = run(x, w_qkv, w_proj, b_proj, trace=False)
    return out_full


# revision 12
# speedup vs baseline: 1.9954x; 1.1842x over previous
"""Multi-head attention block (qkv proj + softmax attention + out proj) on 8
TRN2 NeuronCores, data-parallel over the batch dimension (2 batches/core).

Reference computation (B=16, N=1024, C=1024, H=16, D=64):
    qkv = x @ w_qkv.T                    # [B,N,3C]
    q,k,v per head; attn = softmax(q k^T / sqrt(D)); out = attn @ v
    out = concat_heads @ w_proj.T + b_proj

Device-side design (per core, T = 2*1024 tokens), all channels-on-partitions
("transposed") so the softmax denominator falls out of the PV matmul:
    qk:   qkT[o, t]   = sum_c wqkT[c, o] * xT[c, t]     (q|k channels)
    v:    v[t, vc]    = sum_c xT[c, t] * wvT[c, vc]     (+ ones column/head)
    attn: S_T[key, q] = kT.T @ qT  (K=64, head pairs row-packed on the PE),
          E = exp(S_T * scale) on ACT (no max subtraction needed: |S*scale|<6),
          pv[d|1, q] = [v_h | 1].T @ E  accumulated over key tiles
          -> partition 64 of pv is the denominator;
          normalize: denom -> SBUF, broadcast via K=1 matmul,
          reciprocal_approx_fast, attnT = pv * recip  (fp16)
    out:  out[t, o] = sum_c attnT[c, t] * wpT[c, o] + bias
All matmul inputs fp16 with fp32 PSUM accumulation; PSUM tiles single-bank.

The emission order software-pipelines the whole kernel so the tensor engine
never idles on the ACT exp round-trip (idle PE re-throttles to half clock):
    stage A: qk/v projection chains for batch-0 tokens (dense PE warmup)
    stage B: attention for batch 0, with batch-1 projection chains as filler
    stage C: attention for batch 1, with batch-0 output-proj chains as filler
    stage D: remaining batch-1 output-proj chains
Inside attention units the PV matmuls run one key-tile behind the S matmuls
so the exp latency is hidden by the next tile's S work.
"""

import numpy as np

B, N, C = 16, 1024, 1024
H, D = 16, 64
SCALE = D ** -0.5
NCORES = 8
B_SH = B // NCORES            # batches per core
T = B_SH * N                  # tokens per core
CT = C // 128                 # 128-channel tiles per C
PAIRS = H // 2                # head pairs (2 heads share a 128-row tile)
KT = N // 128                 # key tiles per batch
QB = N // 512                 # q blocks of 512 per batch

_CACHE = {}


def _build():
    import concourse.mybir as mybir
    import concourse.tile as tile
    from concourse import bacc

    fp16 = mybir.dt.float16
    fp32 = mybir.dt.float32
    Exp = mybir.ActivationFunctionType.Exp

    nc = bacc.Bacc("TRN2", target_bir_lowering=False, debug=False)

    xT = nc.dram_tensor("xT", [C, T], fp16, kind="ExternalInput")
    wqkT = nc.dram_tensor("wqkT", [C, 2 * C], fp16, kind="ExternalInput")
    wvT = nc.dram_tensor("wvT", [C, C], fp16, kind="ExternalInput")
    wpT = nc.dram_tensor("wpT", [C, C], fp16, kind="ExternalInput")
    bias = nc.dram_tensor("bias", [128, C], fp32, kind="ExternalInput")
    out = nc.dram_tensor("out", [T, C], fp32, kind="ExternalOutput")
    qkT = nc.dram_tensor("qkT", [2 * C, T], fp16)      # device scratch

    xT_r = xT.rearrange("(j p) t -> p j t", p=128)
    wqkT_r = wqkT.rearrange("(j p) o -> p j o", p=128)
    wvT_r = wvT.rearrange("(j p) o -> p j o", p=128)
    wpT_r = wpT.rearrange("(j p) o -> p j o", p=128)

    with tile.TileContext(nc) as tc:
        with (
            tc.tile_pool(name="persist", bufs=1) as persist,
            tc.tile_pool(name="wstream", bufs=3) as wstream,
            tc.tile_pool(name="evict1", bufs=3) as evict1,
            tc.tile_pool(name="qk2", bufs=4) as qk2,
            tc.tile_pool(name="epool", bufs=6) as epool,
            tc.tile_pool(name="dnpool", bufs=4) as dnpool,
            tc.tile_pool(name="bcpool", bufs=4) as bcpool,
            tc.tile_pool(name="outpool", bufs=4) as outpool,
            tc.tile_pool(name="psum", bufs=1, space="PSUM") as psum,
        ):
            # [tok%128, tok//128, head, d | ones]
            v_ext = persist.tile([128, 2 * KT, H, D + 1], fp16)
            # [c%128, c//128, tok]
            attnT = persist.tile([128, CT, T], fp16)
            x_sb = persist.tile([128, CT, T], fp16)
            wv_sb = persist.tile([128, CT, C], fp16)
            wp_sb = persist.tile([128, CT, C], fp16)
            bias_sb = persist.tile([128, C], fp32)
            ones_sb = persist.tile([128, 64], fp32)
            zero_sb = persist.tile([128, 1], fp32)

            nc.sync.dma_start(x_sb[:], xT_r)
            nc.sync.dma_start(wv_sb[:], wvT_r)
            nc.sync.dma_start(wp_sb[:], wpT_r)
            nc.sync.dma_start(bias_sb[:], bias[:])
            nc.any.memset(ones_sb[:], 1.0)
            nc.any.memset(zero_sb[:], 0.0)
            nc.any.memset(v_ext[:, :, :, D:D + 1], 1.0)

            # -------- projection / output work units (PE filler chains) -----
            def qk_load(ot, stage):
                wq_sb = wstream.tile([128, CT, 128], fp16, tag="wq",
                                     name=f"wq_{ot}_{stage}")
                nc.sync.dma_start(
                    wq_sb[:], wqkT_r[:, :, ot * 128:(ot + 1) * 128])
                return wq_sb

            def qk_unit(ot):
                """qkT[ot*128:(ot+1)*128, :] for half the tokens; which half is
                decided per-chain by the tb list passed in."""
                def emit(tbs, wq_sb=None):
                    if wq_sb is None:
                        wq_sb = qk_load(ot, tbs[0])
                    for tb in tbs:
                        ps = psum.tile([128, 512], fp32, tag="mm", bufs=2,
                                       name=f"qkps_{ot}_{tb}")
                        for ct in range(CT):
                            nc.tensor.matmul(
                                ps[:], wq_sb[:, ct, :],
                                x_sb[:, ct, tb * 512:(tb + 1) * 512],
                                start=(ct == 0), stop=(ct == CT - 1))
                        qk_sb = evict1.tile([128, 512], fp16, tag="qke",
                                            name=f"qke_{ot}_{tb}")
                        nc.vector.tensor_copy(qk_sb[:], ps[:])
                        nc.sync.dma_start(
                            qkT[ot * 128:(ot + 1) * 128,
                                tb * 512:(tb + 1) * 512], qk_sb[:])
                return emit

            def v_chain(tt, ob):
                ps = psum.tile([128, 512], fp32, tag="mm", bufs=2,
                               name=f"vps_{tt}_{ob}")
                for ct in range(CT):
                    nc.tensor.matmul(
                        ps[:], x_sb[:, ct, tt * 128:(tt + 1) * 128],
                        wv_sb[:, ct, ob * 512:(ob + 1) * 512],
                        start=(ct == 0), stop=(ct == CT - 1))
                nc.vector.tensor_copy(
                    v_ext[:, tt, ob * 8:(ob + 1) * 8, 0:D],
                    ps[:].rearrange("p (h d) -> p h d", d=D))

            def out_chain(tt, ob):
                ps = psum.tile([128, 512], fp32, tag="mm", bufs=2,
                               name=f"ops_{tt}_{ob}")
                for j in range(CT):
                    nc.tensor.matmul(
                        ps[:], attnT[:, j, tt * 128:(tt + 1) * 128],
                        wp_sb[:, j, ob * 512:(ob + 1) * 512],
                        start=(j == 0), stop=(j == CT - 1))
                o_sb = outpool.tile([128, 512], fp32, tag="o",
                                    name=f"osb_{tt}_{ob}")
                nc.vector.tensor_add(
                    o_sb[:], ps[:], bias_sb[:, ob * 512:(ob + 1) * 512])
                nc.sync.dma_start(
                    out[tt * 128:(tt + 1) * 128, ob * 512:(ob + 1) * 512],
                    o_sb[:])

            # -------- attention unit (one batch, one head pair) ------------
            def attn_load(b, p):
                q_sb = qk2.tile([128, N], fp16, tag="q", name=f"q_{b}_{p}")
                k_sb = qk2.tile([128, N], fp16, tag="k", name=f"k_{b}_{p}")
                nc.sync.dma_start(
                    q_sb[:], qkT[p * 128:(p + 1) * 128, b * N:(b + 1) * N])
                nc.sync.dma_start(
                    k_sb[:], qkT[C + p * 128:C + (p + 1) * 128,
                                 b * N:(b + 1) * N])
                return q_sb, k_sb

            def attn_unit(b, p, q_sb, k_sb, fillers, max_fill):
                filled = [0]

                def maybe_fill():
                    if fillers and filled[0] < max_fill:
                        fillers.pop()
                        filled[0] += 1
                for qb in range(QB):
                    pv = [psum.tile([D + 1, 512], fp32, tag="pv", bufs=2,
                                    name=f"pv_{b}_{p}_{qb}_{h2}")
                          for h2 in range(2)]
                    e_prev = None
                    for kt in range(KT):
                        # both heads' scores into halves of one 2-bank tile,
                        # one exp over the whole [128, 1024] region
                        s_ps = psum.tile([128, 1024], fp32, tag="s", bufs=2,
                                         name=f"s_{b}_{p}_{qb}_{kt}")
                        for h2 in range(2):
                            ho = h2 * 64
                            nc.tensor.matmul(
                                s_ps[:, h2 * 512:(h2 + 1) * 512],
                                k_sb[ho:ho + 64, kt * 128:(kt + 1) * 128],
                                q_sb[ho:ho + 64, qb * 512:(qb + 1) * 512],
                                start=True, stop=True)
                        e_sb = epool.tile([128, 1024], fp16, tag="e",
                                          name=f"e_{b}_{p}_{qb}_{kt}")
                        nc.scalar.activation(
                            e_sb[:], s_ps[:], Exp,
                            bias=zero_sb[:], scale=SCALE)
                        if kt > 0:
                            for h2 in range(2):
                                nc.tensor.matmul(
                                    pv[h2][:],
                                    v_ext[:, b * KT + kt - 1, 2 * p + h2, :],
                                    e_prev[:, h2 * 512:(h2 + 1) * 512],
                                    start=(kt == 1), stop=False)
                        e_prev = e_sb
                        if kt in (2, 5):
                            maybe_fill()
                    for h2 in range(2):
                        nc.tensor.matmul(
                            pv[h2][:],
                            v_ext[:, b * KT + KT - 1, 2 * p + h2, :],
                            e_prev[:, h2 * 512:(h2 + 1) * 512],
                            start=False, stop=True)
                    for h2 in range(2):
                        dn_raw = dnpool.tile([1, 512], fp32, tag="dnr",
                                             name=f"dnr_{b}_{p}_{qb}_{h2}")
                        nc.vector.tensor_copy(dn_raw[0:1, :], pv[h2][D:D + 1, :])
                        dn = dnpool.tile([1, 512], fp32, tag="dn",
                                         name=f"dn_{b}_{p}_{qb}_{h2}")
                        nc.vector.reciprocal_approx_fast(
                            dn[0:1, :], dn_raw[0:1, :])
                        bc_sb = bcpool.tile([64, 512], fp32, tag="bc",
                                            name=f"bc_{b}_{p}_{qb}_{h2}")
                        nc.gpsimd.partition_broadcast(
                            bc_sb[:], dn[0:1, :], channels=64)
                        tsl = slice(b * N + qb * 512, b * N + (qb + 1) * 512)
                        nc.vector.tensor_mul(
                            attnT[h2 * 64:h2 * 64 + 64, p, tsl],
                            pv[h2][0:D, :], bc_sb[:])
                    maybe_fill()

            # -------------------- emission schedule -----------------------
            class FillerQueue:
                """Filler work as (load_fn, compute_fn) pairs; loads (weight
                DMAs) are issued `depth` items ahead of compute so inserted
                chains never stall the PE on their own DMA."""

                def __init__(self, items, depth=2):
                    self.items = items
                    self.loaded = {}
                    self.next_load = 0
                    self.next_compute = 0
                    self.depth = depth
                    self._pump()

                def _pump(self):
                    while (self.next_load < len(self.items)
                           and self.next_load < self.next_compute + self.depth):
                        load_fn = self.items[self.next_load][0]
                        self.loaded[self.next_load] = (
                            load_fn() if load_fn else None)
                        self.next_load += 1

                def pop(self):
                    if self.next_compute >= len(self.items):
                        return
                    i = self.next_compute
                    self.items[i][1](self.loaded.pop(i))
                    self.next_compute += 1
                    self._pump()

                def __bool__(self):
                    return self.next_compute < len(self.items)

                def flush(self):
                    while self:
                        self.pop()

            # stage A: batch-0 projections (dense PE warmup)
            for ot in range(2 * CT):
                qk_unit(ot)([0, 1])
            for tt in range(KT):
                for ob in range(2):
                    v_chain(tt, ob)

            # stage B filler: batch-1 projections (qk units interleaved with
            # v chains). stage C filler: batch-0 output projection.
            items_b = []
            for i in range(2 * CT):
                items_b.append((lambda ot=i: qk_load(ot, 23),
                                lambda wq, ot=i: qk_unit(ot)([2, 3], wq)))
                if i < KT:
                    items_b.append((None, lambda _, tt=KT + i: v_chain(tt, 0)))
                    items_b.append((None, lambda _, tt=KT + i: v_chain(tt, 1)))
            fq_b = FillerQueue(items_b)
            fq_c = FillerQueue([
                (None, lambda _, tt=tt, ob=ob: out_chain(tt, ob))
                for tt in range(KT) for ob in range(2)])

            units = [(0, p) for p in range(PAIRS)] + [(1, p) for p in range(PAIRS)]
            qk_tiles = {0: attn_load(*units[0])}
            for i, (b, p) in enumerate(units):
                if i + 1 < len(units):
                    qk_tiles[i + 1] = attn_load(*units[i + 1])
                if b == 0:
                    attn_unit(b, p, *qk_tiles.pop(i), fq_b, max_fill=4)
                else:
                    attn_unit(b, p, *qk_tiles.pop(i), fq_c, max_fill=2)
                if i == PAIRS - 1:
                    fq_b.flush()
            fq_c.flush()

            # stage D: batch-1 output projection
            for tt in range(KT, 2 * KT):
                for ob in range(2):
                    out_chain(tt, ob)

    nc.compile()
    return nc


def _get_nc():
    if "nc" not in _CACHE:
        _CACHE["nc"] = _build()
    return _CACHE["nc"]


def _prep_inputs(x, w_qkv, w_proj, b_proj):
    x16 = np.ascontiguousarray(x, dtype=np.float16)
    wq16 = np.asarray(w_qkv, dtype=np.float16)
    wp16 = np.asarray(w_proj, dtype=np.float16)
    wqkT_np = np.ascontiguousarray(wq16[0:2 * C].T)          # [C, 2C]
    wvT_np = np.ascontiguousarray(wq16[2 * C:3 * C].T)       # [C, C]
    wpT_np = np.ascontiguousarray(wp16.T)                    # [C, C]
    bias_np = np.ascontiguousarray(
        np.broadcast_to(np.asarray(b_proj, dtype=np.float32)[None, :], (128, C)))
    in_maps = []
    for core in range(NCORES):
        xs = x16[core * B_SH:(core + 1) * B_SH]              # [B_SH, N, C]
        xT_np = np.ascontiguousarray(xs.transpose(2, 0, 1).reshape(C, T))
        in_maps.append({
            "xT": xT_np, "wqkT": wqkT_np, "wvT": wvT_np,
            "wpT": wpT_np, "bias": bias_np,
        })
    return in_maps


def _install_ntff_hook():
    """The agent image's antenv lacks axon_hooks; synthesize it so
    run_bass_kernel_spmd(trace=True) can capture NTFF profiles."""
    import sys
    import types
    try:
        from antenv.axon_hooks import get_axon_ntff_profile_hook  # noqa: F401
        return
    except ImportError:
        pass
    import antenv
    mod = types.ModuleType("antenv.axon_hooks")
    state = {"hook": None}
    mod.set_axon_ntff_profile_hook = lambda h: state.__setitem__("hook", h)
    mod.get_axon_ntff_profile_hook = lambda: state["hook"]
    sys.modules["antenv.axon_hooks"] = mod
    antenv.axon_hooks = mod
    try:
        from trn_agent_boot.trn_boot import _ntff_profile_via_ctypes
        mod.set_axon_ntff_profile_hook(
            _ntff_profile_via_ctypes("/opt/axon/libaxon_pjrt.so"))
    except Exception as e:  # tracing degrades, run still works
        print("ntff hook install failed:", e)


def run(x, w_qkv, w_proj, b_proj, trace=False):
    """Returns (full_output [B,N,C] fp32, BassKernelResults)."""
    from concourse.bass_utils import run_bass_kernel_spmd

    if trace:
        _install_ntff_hook()
    nc = _get_nc()
    in_maps = _prep_inputs(x, w_qkv, w_proj, b_proj)
    res = run_bass_kernel_spmd(
        nc, in_maps, core_ids=list(range(NCORES)), trace=trace)
    out_full = np.concatenate(
        [r["out"].reshape(B_SH, N, C) for r in res.results], axis=0)
    return out_full.astype(np.float32), res


def kernel(x, w_qkv, w_proj, b_proj):
    out_full, _ = run(x, w_qkv, w_proj, b_proj, trace=False)
    return out_full


# revision 14
# speedup vs baseline: 2.0918x; 1.0483x over previous
"""Multi-head attention block (qkv proj + softmax attention + out proj) on 8
TRN2 NeuronCores, data-parallel over the batch dimension (2 batches/core).

Reference computation (B=16, N=1024, C=1024, H=16, D=64):
    qkv = x @ w_qkv.T                    # [B,N,3C]
    q,k,v per head; attn = softmax(q k^T / sqrt(D)); out = attn @ v
    out = concat_heads @ w_proj.T + b_proj

Device-side design (per core, T = 2*1024 tokens), all channels-on-partitions
("transposed") so the softmax denominator falls out of the PV matmul:
    qk:   qkT[o, t]   = sum_c wqkT[c, o] * xT[c, t]     (q|k channels)
    v:    v[t, vc]    = sum_c xT[c, t] * wvT[c, vc]     (+ ones column/head)
    attn: S_T[key, q] = kT.T @ qT  (K=64, head pairs row-packed on the PE),
          E = exp(S_T * scale) on ACT (no max subtraction needed: |S*scale|<6),
          pv[d|1, q] = [v_h | 1].T @ E  accumulated over key tiles
          -> partition 64 of pv is the denominator;
          normalize: denom -> SBUF, broadcast via K=1 matmul,
          reciprocal_approx_fast, attnT = pv * recip  (fp16)
    out:  out[t, o] = sum_c attnT[c, t] * wpT[c, o] + bias
All matmul inputs fp16 with fp32 PSUM accumulation; PSUM tiles single-bank.

The emission order software-pipelines the whole kernel so the tensor engine
never idles on the ACT exp round-trip (idle PE re-throttles to half clock):
    stage A: qk/v projection chains for batch-0 tokens (dense PE warmup)
    stage B: attention for batch 0, with batch-1 projection chains as filler
    stage C: attention for batch 1, with batch-0 output-proj chains as filler
    stage D: remaining batch-1 output-proj chains
Inside attention units the PV matmuls run one key-tile behind the S matmuls
so the exp latency is hidden by the next tile's S work.
"""

import numpy as np

B, N, C = 16, 1024, 1024
H, D = 16, 64
SCALE = D ** -0.5
NCORES = 8
B_SH = B // NCORES            # batches per core
T = B_SH * N                  # tokens per core
CT = C // 128                 # 128-channel tiles per C
PAIRS = H // 2                # head pairs (2 heads share a 128-row tile)
KT = N // 128                 # key tiles per batch
QB = N // 512                 # q blocks of 512 per batch

_CACHE = {}


def _build():
    import concourse.mybir as mybir
    import concourse.tile as tile
    from concourse import bacc

    fp16 = mybir.dt.float16
    fp32 = mybir.dt.float32
    Exp = mybir.ActivationFunctionType.Exp

    nc = bacc.Bacc("TRN2", target_bir_lowering=False, debug=False)

    xT = nc.dram_tensor("xT", [C, T], fp16, kind="ExternalInput")
    wqkT = nc.dram_tensor("wqkT", [C, 2 * C], fp16, kind="ExternalInput")
    wvT = nc.dram_tensor("wvT", [C, C], fp16, kind="ExternalInput")
    wpT = nc.dram_tensor("wpT", [C, C], fp16, kind="ExternalInput")
    bias = nc.dram_tensor("bias", [128, C], fp32, kind="ExternalInput")
    out = nc.dram_tensor("out", [T, C], fp32, kind="ExternalOutput")
    qkT = nc.dram_tensor("qkT", [2 * C, T], fp16)      # device scratch

    xT_r = xT.rearrange("(j p) t -> p j t", p=128)
    wqkT_r = wqkT.rearrange("(j p) o -> p j o", p=128)
    wvT_r = wvT.rearrange("(j p) o -> p j o", p=128)
    wpT_r = wpT.rearrange("(j p) o -> p j o", p=128)

    with tile.TileContext(nc) as tc:
        with (
            tc.tile_pool(name="persist", bufs=1) as persist,
            tc.tile_pool(name="wstream", bufs=4) as wstream,
            tc.tile_pool(name="evict1", bufs=3) as evict1,
            tc.tile_pool(name="qk2", bufs=4) as qk2,
            tc.tile_pool(name="epool", bufs=6) as epool,
            tc.tile_pool(name="dnpool", bufs=4) as dnpool,
            tc.tile_pool(name="bcpool", bufs=4) as bcpool,
            tc.tile_pool(name="outpool", bufs=4) as outpool,
            tc.tile_pool(name="psum", bufs=1, space="PSUM") as psum,
        ):
            # [tok%128, tok//128, head, d | ones]
            v_ext = persist.tile([128, 2 * KT, H, D + 1], fp16)
            # [c%128, c//128, tok]
            attnT = persist.tile([128, CT, T], fp16)
            x_sb = [persist.tile([128, T], fp16, name=f"x{ct}")
                    for ct in range(CT)]
            wv_sb = persist.tile([128, CT, C], fp16)
            wp_sb = persist.tile([128, CT, C], fp16)
            bias_sb = persist.tile([128, C], fp32)
            ones_sb = persist.tile([128, 64], fp32)
            zero_sb = persist.tile([128, 1], fp32)

            for ct in range(CT):
                nc.sync.dma_start(x_sb[ct][:], xT_r[:, ct, :])
            nc.any.memset(ones_sb[:], 1.0)
            nc.any.memset(zero_sb[:], 0.0)
            nc.any.memset(v_ext[:, :, :, D:D + 1], 1.0)

            # -------- projection / output work units (PE filler chains) -----
            def qk_load(ot, stage):
                wq_sb = wstream.tile([128, CT, 128], fp16, tag="wq",
                                     name=f"wq_{ot}_{stage}")
                nc.sync.dma_start(
                    wq_sb[:], wqkT_r[:, :, ot * 128:(ot + 1) * 128])
                return wq_sb

            def qk_unit(ot):
                """qkT[ot*128:(ot+1)*128, :] for half the tokens; which half is
                decided per-chain by the tb list passed in."""
                def emit(tbs, wq_sb=None):
                    if wq_sb is None:
                        wq_sb = qk_load(ot, tbs[0])
                    for tb in tbs:
                        ps = psum.tile([128, 512], fp32, tag="mm", bufs=2,
                                       name=f"qkps_{ot}_{tb}")
                        for ct in range(CT):
                            nc.tensor.matmul(
                                ps[:], wq_sb[:, ct, :],
                                x_sb[ct][:, tb * 512:(tb + 1) * 512],
                                start=(ct == 0), stop=(ct == CT - 1))
                        qk_sb = evict1.tile([128, 512], fp16, tag="qke",
                                            name=f"qke_{ot}_{tb}")
                        nc.vector.tensor_copy(qk_sb[:], ps[:])
                        nc.sync.dma_start(
                            qkT[ot * 128:(ot + 1) * 128,
                                tb * 512:(tb + 1) * 512], qk_sb[:])
                return emit

            def v_chain(tt, ob):
                ps = psum.tile([128, 512], fp32, tag="mm", bufs=2,
                               name=f"vps_{tt}_{ob}")
                for ct in range(CT):
                    nc.tensor.matmul(
                        ps[:], x_sb[ct][:, tt * 128:(tt + 1) * 128],
                        wv_sb[:, ct, ob * 512:(ob + 1) * 512],
                        start=(ct == 0), stop=(ct == CT - 1))
                nc.vector.tensor_copy(
                    v_ext[:, tt, ob * 8:(ob + 1) * 8, 0:D],
                    ps[:].rearrange("p (h d) -> p h d", d=D))

            def out_chain(tt, ob):
                ps = psum.tile([128, 512], fp32, tag="mm", bufs=2,
                               name=f"ops_{tt}_{ob}")
                for j in range(CT):
                    nc.tensor.matmul(
                        ps[:], attnT[:, j, tt * 128:(tt + 1) * 128],
                        wp_sb[:, j, ob * 512:(ob + 1) * 512],
                        start=(j == 0), stop=(j == CT - 1))
                o_sb = outpool.tile([128, 512], fp32, tag="o",
                                    name=f"osb_{tt}_{ob}")
                nc.vector.tensor_add(
                    o_sb[:], ps[:], bias_sb[:, ob * 512:(ob + 1) * 512])
                nc.sync.dma_start(
                    out[tt * 128:(tt + 1) * 128, ob * 512:(ob + 1) * 512],
                    o_sb[:])

            # -------- attention unit (one batch, one head pair) ------------
            def attn_load(b, p):
                q_sb = qk2.tile([128, N], fp16, tag="q", name=f"q_{b}_{p}")
                k_sb = qk2.tile([128, N], fp16, tag="k", name=f"k_{b}_{p}")
                nc.sync.dma_start(
                    q_sb[:], qkT[p * 128:(p + 1) * 128, b * N:(b + 1) * N])
                nc.sync.dma_start(
                    k_sb[:], qkT[C + p * 128:C + (p + 1) * 128,
                                 b * N:(b + 1) * N])
                return q_sb, k_sb

            def attn_unit(b, p, q_sb, k_sb, fillers, max_fill):
                filled = [0]

                def maybe_fill():
                    if fillers and filled[0] < max_fill:
                        fillers.pop()
                        filled[0] += 1
                for qb in range(QB):
                    pv = [psum.tile([D + 1, 512], fp32, tag="pv", bufs=2,
                                    name=f"pv_{b}_{p}_{qb}_{h2}")
                          for h2 in range(2)]
                    e_prev = None
                    for kt in range(KT):
                        # both heads' scores into halves of one 2-bank tile,
                        # one exp over the whole [128, 1024] region
                        s_ps = psum.tile([128, 1024], fp32, tag="s", bufs=2,
                                         name=f"s_{b}_{p}_{qb}_{kt}")
                        for h2 in range(2):
                            ho = h2 * 64
                            nc.tensor.matmul(
                                s_ps[:, h2 * 512:(h2 + 1) * 512],
                                k_sb[ho:ho + 64, kt * 128:(kt + 1) * 128],
                                q_sb[ho:ho + 64, qb * 512:(qb + 1) * 512],
                                start=True, stop=True)
                        e_sb = epool.tile([128, 1024], fp16, tag="e",
                                          name=f"e_{b}_{p}_{qb}_{kt}")
                        nc.scalar.activation(
                            e_sb[:], s_ps[:], Exp,
                            bias=zero_sb[:], scale=SCALE)
                        if kt > 0:
                            for h2 in range(2):
                                nc.tensor.matmul(
                                    pv[h2][:],
                                    v_ext[:, b * KT + kt - 1, 2 * p + h2, :],
                                    e_prev[:, h2 * 512:(h2 + 1) * 512],
                                    start=(kt == 1), stop=False)
                        e_prev = e_sb
                        if kt in (2, 5):
                            maybe_fill()
                    for h2 in range(2):
                        nc.tensor.matmul(
                            pv[h2][:],
                            v_ext[:, b * KT + KT - 1, 2 * p + h2, :],
                            e_prev[:, h2 * 512:(h2 + 1) * 512],
                            start=False, stop=True)
                    for h2 in range(2):
                        dn_raw = dnpool.tile([1, 512], fp32, tag="dnr",
                                             name=f"dnr_{b}_{p}_{qb}_{h2}")
                        nc.vector.tensor_copy(dn_raw[0:1, :], pv[h2][D:D + 1, :])
                        dn = dnpool.tile([1, 512], fp32, tag="dn",
                                         name=f"dn_{b}_{p}_{qb}_{h2}")
                        nc.vector.reciprocal_approx_fast(
                            dn[0:1, :], dn_raw[0:1, :])
                        bc_sb = bcpool.tile([64, 512], fp32, tag="bc",
                                            name=f"bc_{b}_{p}_{qb}_{h2}")
                        nc.gpsimd.partition_broadcast(
                            bc_sb[:], dn[0:1, :], channels=64)
                        tsl = slice(b * N + qb * 512, b * N + (qb + 1) * 512)
                        nc.vector.tensor_mul(
                            attnT[h2 * 64:h2 * 64 + 64, p, tsl],
                            pv[h2][0:D, :], bc_sb[:])
                    maybe_fill()

            # -------------------- emission schedule -----------------------
            class FillerQueue:
                """Filler work as (load_fn, compute_fn) pairs; loads (weight
                DMAs) are issued `depth` items ahead of compute so inserted
                chains never stall the PE on their own DMA."""

                def __init__(self, items, depth=2):
                    self.items = items
                    self.loaded = {}
                    self.next_load = 0
                    self.next_compute = 0
                    self.depth = depth
                    self._pump()

                def _pump(self):
                    while (self.next_load < len(self.items)
                           and self.next_load < self.next_compute + self.depth):
                        load_fn = self.items[self.next_load][0]
                        self.loaded[self.next_load] = (
                            load_fn() if load_fn else None)
                        self.next_load += 1

                def pop(self):
                    if self.next_compute >= len(self.items):
                        return
                    i = self.next_compute
                    self.items[i][1](self.loaded.pop(i))
                    self.next_compute += 1
                    self._pump()

                def __bool__(self):
                    return self.next_compute < len(self.items)

                def flush(self):
                    while self:
                        self.pop()

            # stage A: batch-0 projections (dense PE warmup)
            for ot in range(2 * CT):
                qk_unit(ot)([0, 1])
                if ot == 1:
                    nc.sync.dma_start(wv_sb[:], wvT_r)
                    nc.sync.dma_start(wp_sb[:], wpT_r)
                    nc.sync.dma_start(bias_sb[:], bias[:])
            for tt in range(KT):
                for ob in range(2):
                    v_chain(tt, ob)

            # stage B filler: batch-1 qk projection for pairs 0-3 plus all
            # batch-1 v chains. stage C filler: batch-1 qk projection for
            # pairs 4-7 (popped early enough for the prefetched loads) plus
            # batch-0 output projection.
            def qk_item(ot):
                return (lambda: qk_load(ot, 23),
                        lambda wq: qk_unit(ot)([2, 3], wq))

            items_b = []
            for j in range(4):
                items_b.append(qk_item(j))
                items_b.append((None, lambda _, tt=KT + 2 * j: v_chain(tt, 0)))
                items_b.append((None, lambda _, tt=KT + 2 * j: v_chain(tt, 1)))
                items_b.append(qk_item(CT + j))
                items_b.append((None, lambda _, tt=KT + 2 * j + 1: v_chain(tt, 0)))
                items_b.append((None, lambda _, tt=KT + 2 * j + 1: v_chain(tt, 1)))
            fq_b = FillerQueue(items_b)

            items_c = []
            outs_c = [(tt, ob) for tt in range(KT) for ob in range(2)]
            for j in range(4):
                items_c.append(qk_item(4 + j))
                items_c.append(qk_item(CT + 4 + j))
                tt, ob = outs_c.pop(0)
                items_c.append((None, lambda _, tt=tt, ob=ob: out_chain(tt, ob)))
            for tt, ob in outs_c:
                items_c.append((None, lambda _, tt=tt, ob=ob: out_chain(tt, ob)))
            fq_c = None  # created lazily at the stage-C transition so its
            #              prefetched weight loads don't pin pool slots early

            units = [(0, p) for p in range(PAIRS)] + [(1, p) for p in range(PAIRS)]
            qk_tiles = {0: attn_load(*units[0])}
            for i, (b, p) in enumerate(units):
                if i + 1 < len(units):
                    qk_tiles[i + 1] = attn_load(*units[i + 1])
                if b == 0:
                    attn_unit(b, p, *qk_tiles.pop(i), fq_b, max_fill=3)
                else:
                    if fq_c is None:
                        fq_c = FillerQueue(items_c)
                    attn_unit(b, p, *qk_tiles.pop(i), fq_c, max_fill=3)
                if i == PAIRS - 1:
                    fq_b.flush()
            fq_c.flush()

            # stage D: batch-1 output projection
            for tt in range(KT, 2 * KT):
                for ob in range(2):
                    out_chain(tt, ob)

    nc.compile()
    return nc


def _get_nc():
    if "nc" not in _CACHE:
        _CACHE["nc"] = _build()
    return _CACHE["nc"]


def _prep_inputs(x, w_qkv, w_proj, b_proj):
    x16 = np.ascontiguousarray(x, dtype=np.float16)
    wq16 = np.asarray(w_qkv, dtype=np.float16)
    wp16 = np.asarray(w_proj, dtype=np.float16)
    wqkT_np = np.ascontiguousarray(wq16[0:2 * C].T)          # [C, 2C]
    wvT_np = np.ascontiguousarray(wq16[2 * C:3 * C].T)       # [C, C]
    wpT_np = np.ascontiguousarray(wp16.T)                    # [C, C]
    bias_np = np.ascontiguousarray(
        np.broadcast_to(np.asarray(b_proj, dtype=np.float32)[None, :], (128, C)))
    in_maps = []
    for core in range(NCORES):
        xs = x16[core * B_SH:(core + 1) * B_SH]              # [B_SH, N, C]
        xT_np = np.ascontiguousarray(xs.transpose(2, 0, 1).reshape(C, T))
        in_maps.append({
            "xT": xT_np, "wqkT": wqkT_np, "wvT": wvT_np,
            "wpT": wpT_np, "bias": bias_np,
        })
    return in_maps


def _install_ntff_hook():
    """The agent image's antenv lacks axon_hooks; synthesize it so
    run_bass_kernel_spmd(trace=True) can capture NTFF profiles."""
    import sys
    import types
    try:
        from antenv.axon_hooks import get_axon_ntff_profile_hook  # noqa: F401
        return
    except ImportError:
        pass
    import antenv
    mod = types.ModuleType("antenv.axon_hooks")
    state = {"hook": None}
    mod.set_axon_ntff_profile_hook = lambda h: state.__setitem__("hook", h)
    mod.get_axon_ntff_profile_hook = lambda: state["hook"]
    sys.modules["antenv.axon_hooks"] = mod
    antenv.axon_hooks = mod
    try:
        from trn_agent_boot.trn_boot import _ntff_profile_via_ctypes
        mod.set_axon_ntff_profile_hook(
            _ntff_profile_via_ctypes("/opt/axon/libaxon_pjrt.so"))
    except Exception as e:  # tracing degrades, run still works
        print("ntff hook install failed:", e)


def run(x, w_qkv, w_proj, b_proj, trace=False):
    """Returns (full_output [B,N,C] fp32, BassKernelResults)."""
    from concourse.bass_utils import run_bass_kernel_spmd

    if trace:
        _install_ntff_hook()
    nc = _get_nc()
    in_maps = _prep_inputs(x, w_qkv, w_proj, b_proj)
    res = run_bass_kernel_spmd(
        nc, in_maps, core_ids=list(range(NCORES)), trace=trace)
    out_full = np.concatenate(
        [r["out"].reshape(B_SH, N, C) for r in res.results], axis=0)
    return out_full.astype(np.float32), res


def kernel(x, w_qkv, w_proj, b_proj):
    out_full, _ = run(x, w_qkv, w_proj, b_proj, trace=False)
    return out_full
